# revision 18
# baseline (speedup 1.0000x reference)
"""Trainium2 Bass kernel for SSD-style DetectionLoss (nms_detection).

kernel(**inputs) takes FULL inputs (bbox_pred [32,32768,4], conf_pred
[32,32768], anchors [32768,4], gt_boxes [32,64,4]) and returns the full output
(loc+conf loss, conf loss, loc loss) as fp32 scalars.

Sharding: data-parallel over batch - each of 8 NeuronCores gets 4 images; the
host sums the per-core partials (loc, pos-conf, neg-conf, num_pos): the
cross-core all-reduce is 4 scalars, done in the gather step.

Per-core layout: partitions = (2 images) x (64 gts) = 128, free = anchors in
16 chunks of 2048.  Anchor rows are broadcast to all partitions by
stride-0-source DMAs per chunk/coord; gt coords are per-partition scalars.

IoU width/height use the relu identity
    min(ax2,gx2) - max(ax1,gx1) = (gx2-gx1) - relu(gx2-ax2) - relu(ax1-gx1)
so the whole clipped-width chain runs as activation ops (per-partition bias,
scale=+-1) on the otherwise-idle ACT engine; the two relu-sums and inter run
as plain tensor_tensor on GPSIMD (fast path; tensor_scalar on GPSIMD is ~7x
slower).  1/union uses the single-pass approx reciprocal (~51 ULP; the NR
refinement is ~12x the cost for nothing at this tolerance).  iou and its
per-gt max come from one fused tensor_tensor_reduce.  Per-anchor rowmax goes
through PE transposes + segmented reduce; matched-gt coords are gathered with
a (iou == rowmax) one-hot contracted against the gt table on the PE; the
forced best-anchor-per-gt is recovered from per-chunk colmax + index dot
products + a one-hot-vs-iota PE contraction.  Hard-negative mining (sum of
top-num_neg negative BCE values) uses a 10-round 4-ary threshold bisection +
boundary-count correction, with counts split between ACT (sign trick) and DVE.
"""

import numpy as np

B, A, G = 32, 32768, 64
N_CORES = 8
IMGS = B // N_CORES          # images per core
R = IMGS // 2                # image pairs per core
CH = 2048                    # anchors per chunk
NCHUNK = A // CH             # 16
F = A // 128                 # a = f*128 + p
FC = F // NCHUNK             # f-blocks (=128-anchor blocks) per chunk = 16
NEG_POS_RATIO = 3.0
EPS = 1e-6
BIS_ROUNDS = 10
BIS_RANGE = 9.3

_CACHE = {}


def _build_program():
    import concourse.bacc as bacc
    import concourse.mybir as mybir
    import concourse.bass_isa as bass_isa
    from concourse import tile
    from concourse.bass_types import AP
    from concourse.masks import make_identity
    import contextlib

    dt = mybir.dt
    Alu = mybir.AluOpType
    Act = mybir.ActivationFunctionType
    X = mybir.AxisListType.X

    nc = bacc.Bacc(None, target_bir_lowering=False, debug=False)

    def bcast_src(dram_tile, off_elems, n):
        ap = dram_tile[:]
        return AP(tensor=ap.tensor, offset=ap.offset + off_elems,
                  ap=[[1, 1], [0, 128], [1, n]])

    with tile.TileContext(nc) as tc:
        ctx = contextlib.ExitStack()
        dram = ctx.enter_context(tc.tile_pool(name="dram", bufs=1, space="DRAM"))
        consts = ctx.enter_context(tc.tile_pool(name="consts", bufs=1))
        pers = ctx.enter_context(tc.tile_pool(name="pers", bufs=1))
        work = ctx.enter_context(tc.tile_pool(name="work", bufs=1))
        work2 = ctx.enter_context(tc.tile_pool(name="work2", bufs=2))
        psA = ctx.enter_context(tc.tile_pool(name="psA", bufs=2, space="PSUM"))
        psB = ctx.enter_context(tc.tile_pool(name="psB", bufs=1, space="PSUM"))

        # ---------------- DRAM I/O ----------------
        anch_rows = dram.tile([4, A], dt.float32, kind="ExternalInput", name="anch_rows")
        anch_ap = dram.tile([128, 4, F], dt.float32, kind="ExternalInput", name="anch_ap")
        gt_cols = dram.tile([R, 128, 4], dt.float32, kind="ExternalInput", name="gt_cols")
        bbox_in = dram.tile([128, R, F, 2, 4], dt.float32, kind="ExternalInput", name="bbox_ap")
        conf_in = dram.tile([128, R, F, 2], dt.float32, kind="ExternalInput", name="conf_ap")
        out_d = dram.tile([1, 8], dt.float32, kind="ExternalOutput", name="part_out")
        area_d = dram.tile([1, A], dt.float32, kind="Internal", name="area_row")
        rmscr_d = dram.tile([R, NCHUNK, 2 * CH], dt.float32, kind="Internal",
                            name="rm_scr")

        # ---------------- persistent state ----------------
        matched = pers.tile([128, R, F, 8], dt.float32)
        rm = pers.tile([128, R, NCHUNK, 2, FC], dt.float32)
        state = pers.tile([128, 3, 1024], dt.float32)
        s2 = state[:, 2, :]
        cmax = s2[:, 0:32].rearrange("p (r c) -> p r c", r=R)
        aix = s2[:, 32:64].rearrange("p (r c) -> p r c", r=R)
        fcnt = state[:, 1, :].rearrange("p (r c t i) -> p r c t i", r=R, c=NCHUNK, t=FC)

        # ---------------- work tiles (fixed tags, reused across phases) ----
        tpx = work.tile([128, CH], dt.float32, tag="t_px")
        tqx = work.tile([128, CH], dt.float32, tag="t_qx")
        tpy = work.tile([128, CH], dt.float32, tag="t_py")
        tqy = work.tile([128, CH], dt.float32, tag="t_qy")
        taga = work.tile([128, CH], dt.float32, tag="t_aga")
        tiou = work.tile([128, CH], dt.float32, tag="t_iou")
        teq = work.tile([128, CH], dt.float32, tag="t_eq")
        rmts2 = work.tile([2, CH], dt.float32, tag="t_rm2")
        conf_sb = work.tile([128, R, F, 2], dt.float32, tag="t_conf")

        # prefetch conf early (overlaps the main loop)
        nc.sync.dma_start(conf_sb[:], conf_in[:])

        # ---------------- constants ----------------
        ident = consts.tile([128, 128], dt.float32)
        make_identity(nc, ident[:])
        ones128 = consts.tile([128, 128], dt.float32)
        nc.vector.memset(ones128[:], 1.0)
        ind2 = consts.tile([2, 128], dt.float32)
        ind2_x = work.tile([2, 128], dt.int32, tag="t_px")
        ind2_y = work.tile([2, 128], dt.int32, tag="t_qx")
        nc.gpsimd.iota(ind2_x[:], pattern=[[1, 128]], base=0, channel_multiplier=0)
        nc.gpsimd.iota(ind2_y[:], pattern=[[0, 128]], base=0, channel_multiplier=1)
        ind2_i = work.tile([2, 128], dt.int32, tag="t_py")
        nc.vector.tensor_scalar(out=ind2_i[:], in0=ind2_x[:], scalar1=6, scalar2=None,
                                op0=Alu.arith_shift_right)
        nc.vector.tensor_tensor(out=ind2_i[:], in0=ind2_i[:], in1=ind2_y[:],
                                op=Alu.is_equal)
        nc.vector.tensor_copy(ind2[:], ind2_i[:])
        indc = consts.tile([128, 2], dt.float32)
        nc.vector.memset(indc[:], 0.0)
        nc.vector.memset(indc[0:64, 0:1], 1.0)
        nc.vector.memset(indc[64:128, 1:2], 1.0)
        iotaf = consts.tile([128, CH], dt.float32)
        iot_i = work.tile([128, CH], dt.int32, tag="t_px")
        nc.gpsimd.iota(iot_i[:], pattern=[[1, CH]], base=0, channel_multiplier=0)
        nc.vector.tensor_copy(iotaf[:], iot_i[:])
        offs16 = consts.tile([128, NCHUNK], dt.float32)
        offs_i = work.tile([128, NCHUNK], dt.int32, tag="t_qx")
        nc.gpsimd.iota(offs_i[:], pattern=[[CH, NCHUNK]], base=0, channel_multiplier=0)
        nc.vector.tensor_copy(offs16[:], offs_i[:])

        gtc, ngx1, ngy1, ga, gw, gh, rhs_gt = [], [], [], [], [], [], []
        for r in range(R):
            g = consts.tile([128, 4], dt.float32, tag=f"gtc{r}")
            nc.sync.dma_start(g[:], gt_cols[r])
            gtc.append(g)
            nx = consts.tile([128, 1], dt.float32, tag=f"ngx{r}")
            nc.vector.tensor_scalar_mul(nx[:], g[:, 0:1], -1.0)
            ngx1.append(nx)
            ny = consts.tile([128, 1], dt.float32, tag=f"ngy{r}")
            nc.vector.tensor_scalar_mul(ny[:], g[:, 1:2], -1.0)
            ngy1.append(ny)
            w_ = consts.tile([128, 1], dt.float32, tag=f"gw{r}")
            nc.vector.tensor_tensor(out=w_[:], in0=g[:, 2:3], in1=g[:, 0:1], op=Alu.subtract)
            gw.append(w_)
            h_ = consts.tile([128, 1], dt.float32, tag=f"gh{r}")
            nc.vector.tensor_tensor(out=h_[:], in0=g[:, 3:4], in1=g[:, 1:2], op=Alu.subtract)
            gh.append(h_)
            gar = consts.tile([128, 1], dt.float32, tag=f"ga{r}")
            nc.vector.tensor_tensor(out=gar[:], in0=w_[:], in1=h_[:], op=Alu.mult)
            nc.vector.tensor_scalar_add(gar[:], gar[:], EPS)
            ga.append(gar)
            rg = consts.tile([128, 8], dt.float32, tag=f"rhs{r}")
            nc.vector.memset(rg[:], 0.0)
            nc.vector.tensor_copy(rg[0:64, 0:4], g[0:64, :])
            nc.vector.tensor_copy(rg[64:128, 4:8], g[64:128, :])
            rhs_gt.append(rg)

        # ---------------- anchor area -> DRAM row ----------------
        aA = work.tile([128, 4, F], dt.float32, tag="t_eq")  # reuse teq space
        nc.sync.dma_start(aA[:], anch_ap[:])
        awid = tpy
        nc.vector.tensor_tensor(out=awid[:, 0:F], in0=aA[:, 2], in1=aA[:, 0],
                                op=Alu.subtract)
        ahei = tqy
        nc.vector.tensor_tensor(out=ahei[:, 0:F], in0=aA[:, 3], in1=aA[:, 1],
                                op=Alu.subtract)
        area_a = taga
        nc.vector.tensor_tensor(out=area_a[:, 0:F], in0=awid[:, 0:F],
                                in1=ahei[:, 0:F], op=Alu.mult)
        ad_ap = area_d[:]
        dst = AP(tensor=ad_ap.tensor, offset=ad_ap.offset, ap=[[1, 1], [1, 128], [128, F]])
        nc.sync.dma_start(dst, area_a[:, 0:F])

        # ================= main loop =================
        for c in range(NCHUNK):
            bts = []
            for t in range(4):
                bt = work2.tile([128, CH], dt.float32, tag=f"bt{t}")
                nc.sync.dma_start(bt[:], bcast_src(anch_rows, t * A + c * CH, CH))
                bts.append(bt)
            bx1, by1, bx2, by2 = bts
            bar = work2.tile([128, CH], dt.float32, tag="bt4")
            nc.sync.dma_start(bar[:], bcast_src(area_d, c * CH, CH))

            for r in range(R):
                k = c * R + r
                ti = work.tile([128, CH], dt.float32,
                               tag=["t_px", "t_eq"][k % 2])
                # clipped width/height via relu identity, all on ACT
                nc.scalar.activation(ti[:], bx2[:], Act.Relu,
                                     bias=gtc[r][:, 2:3], scale=-1.0)
                nc.scalar.activation(tqx[:], bx1[:], Act.Relu,
                                     bias=ngx1[r][:], scale=1.0)
                nc.scalar.activation(tpy[:], by2[:], Act.Relu,
                                     bias=gtc[r][:, 3:4], scale=-1.0)
                nc.scalar.activation(tqy[:], by1[:], Act.Relu,
                                     bias=ngy1[r][:], scale=1.0)
                nc.gpsimd.tensor_tensor(out=ti[:], in0=ti[:], in1=tqx[:], op=Alu.add)
                nc.gpsimd.tensor_tensor(out=tpy[:], in0=tpy[:], in1=tqy[:], op=Alu.add)
                nc.scalar.activation(tqx[:], ti[:], Act.Relu,
                                     bias=gw[r][:], scale=-1.0)
                nc.scalar.activation(tqy[:], tpy[:], Act.Relu,
                                     bias=gh[r][:], scale=-1.0)
                # inter on GPSIMD, union/recip/iou on DVE
                nc.gpsimd.tensor_tensor(out=ti[:], in0=tqx[:], in1=tqy[:], op=Alu.mult)
                nc.vector.tensor_scalar(out=taga[:], in0=bar[:], scalar1=ga[r][:],
                                        scalar2=None, op0=Alu.add)
                nc.vector.tensor_tensor(out=taga[:], in0=taga[:], in1=ti[:],
                                        op=Alu.subtract)
                nc.vector.reciprocal_approx_fast(out=tqy[:], in_=taga[:])
                nc.vector.tensor_tensor(out=tiou[:], in0=ti[:], in1=tqy[:],
                                        op=Alu.mult)
                nc.vector.tensor_reduce(out=cmax[:, r, c:c + 1], in_=tiou[:],
                                        axis=X, op=Alu.max)
                # argmax-over-anchors accumulator (per gt; scratch out -> taga,
                # already dead, so tpy stays free for the next iteration's ACT)
                nc.vector.scalar_tensor_tensor(out=taga[:], in0=tiou[:],
                                               scalar=cmax[:, r, c:c + 1],
                                               in1=iotaf[:],
                                               op0=Alu.is_ge, op1=Alu.mult,
                                               accum_out=aix[:, r, c:c + 1])
                # per-anchor rowmax: PE transposes + segmented reduce
                for tg in range(4):
                    tp = psA.tile([128, 512], dt.float32, tag="tp")
                    for t4 in range(4):
                        t = tg * 4 + t4
                        nc.tensor.transpose(tp[:, t4 * 128:(t4 + 1) * 128],
                                            tiou[:, t * 128:(t + 1) * 128], ident[:])
                    nc.vector.tensor_reduce(
                        out=rm[:, r, c, :, 4 * tg:4 * tg + 4].rearrange(
                            "p i t -> p t i"),
                        in_=tp[:].rearrange("p (t i g) -> p t i g", t=4, i=2),
                        axis=X, op=Alu.max)
                # rm[:, r, c] (p-part, i, t) -> DRAM (p*32+i*16+t) -> [2, CH] rows
                nc.sync.dma_start(
                    rmscr_d[r, c].rearrange("(p i t) -> p i t", p=128, i=2),
                    rm[:, r, c])
                scr_ap = rmscr_d[r, c]
                src_v = AP(tensor=scr_ap.tensor, offset=scr_ap.offset,
                           ap=[[FC, 2], [2 * FC, 128], [1, FC]])
                nc.sync.dma_start(
                    rmts2[:].rearrange("i (u t) -> i u t", t=FC), src_v)
                rmb = psB.tile([128, CH], dt.float32, tag="rmbp")
                rhs_v = rmts2[:].rearrange("i (u t) -> i t u", t=FC)
                for h in range(4):
                    nc.tensor.matmul(rmb[:, h * 512:(h + 1) * 512], ind2[:],
                                     rhs_v[:, 4 * h:4 * h + 4, :],
                                     start=True, stop=True)
                nc.vector.tensor_tensor(out=tiou[:], in0=tiou[:], in1=rmb[:],
                                        op=Alu.is_equal)
                mm = psB.tile([128, FC, 8], dt.float32, tag="mmp")
                for t in range(FC):
                    nc.tensor.matmul(mm[:, t, :], tiou[:, t * 128:(t + 1) * 128],
                                     rhs_gt[r][:], start=True, stop=True)
                nc.vector.tensor_copy(matched[:, r, c * FC:(c + 1) * FC, :], mm[:])

        # ================= forced anchors =================
        gmax = s2[:, 64:66]
        nc.vector.tensor_reduce(out=gmax, in_=cmax, axis=X, op=Alu.max)
        gaidx = s2[:, 66:68]
        for r in range(R):
            sel = s2[:, 100:116]
            nc.vector.tensor_scalar(out=sel, in0=cmax[:, r, :], scalar1=gmax[:, r:r + 1],
                                    scalar2=None, op0=Alu.is_ge)
            axo = s2[:, 116:132]
            nc.vector.tensor_tensor(out=axo, in0=aix[:, r, :], in1=offs16[:], op=Alu.add)
            scr2 = s2[:, 132:148]
            nc.vector.scalar_tensor_tensor(out=scr2, in0=sel, scalar=1.0, in1=axo,
                                           op0=Alu.mult, op1=Alu.mult,
                                           accum_out=gaidx[:, r:r + 1])

        # ================= bce log terms (ACT; overlaps forced-anchor PE) ===
        cs_flat = conf_sb[:].rearrange("p r f i -> p (r f i)")
        logp = tpx[:, 0:1024]
        nc.scalar.activation(logp, cs_flat, Act.Ln, bias=0.0, scale=1.0)
        l1m = tqx[:, 0:1024]
        nc.scalar.activation(l1m, cs_flat, Act.Ln, bias=1.0, scale=-1.0)

        # ================= loc smooth-L1 per anchor (needs matched, not pos)
        # vsl = l_ + m_*(0.5*q - l_)  ==  where(|d|<1, 0.5 d^2, |d|-0.5)
        from concourse.bass_types import AP as _AP
        vsl_r = []
        bb_tags = ["t_iou", "t_rm2"]
        for r in range(R):
            bb = work.tile([128, F, 2, 4], dt.float32, tag=bb_tags[r])
            nc.sync.dma_start(bb[:], bbox_in[:, r])
            e_t = work.tile([128, 4, 2, F], dt.float32, tag="t_eq")
            nc.vector.tensor_tensor(
                out=e_t[:].rearrange("p c i f -> p f i c"),
                in0=bb[:],
                in1=matched[:, r].rearrange("p f (i c) -> p f i c", i=2),
                op=Alu.subtract)
            d4 = work.tile([128, 4, 2, F], dt.float32, tag="t_py")
            nc.vector.tensor_tensor(out=d4[:, 0], in0=e_t[:, 0], in1=e_t[:, 2], op=Alu.add)
            nc.vector.tensor_tensor(out=d4[:, 1], in0=e_t[:, 1], in1=e_t[:, 3], op=Alu.add)
            d01 = d4[:].rearrange("p c i f -> p (c i f)")[:, 0:1024]
            nc.vector.tensor_scalar_mul(d01, d01, 0.5)
            nc.vector.tensor_tensor(out=d4[:, 2], in0=e_t[:, 2], in1=e_t[:, 0], op=Alu.subtract)
            nc.vector.tensor_tensor(out=d4[:, 3], in0=e_t[:, 3], in1=e_t[:, 1], op=Alu.subtract)
            d4f = d4[:].rearrange("p c i f -> p (c i f)")
            ad = work.tile([128, 2048], dt.float32, tag="t_qy")
            nc.vector.tensor_scalar(out=ad[:].bitcast(dt.int32),
                                    in0=d4f.bitcast(dt.int32),
                                    scalar1=0x7FFFFFFF, scalar2=None,
                                    op0=Alu.bitwise_and)
            q = work.tile([128, 2048], dt.float32, tag="t_aga")
            nc.gpsimd.tensor_tensor(out=q[:], in0=d4f, in1=d4f, op=Alu.mult)
            m_ = work.tile([128, 2048], dt.float32, tag="t_eq")
            nc.vector.tensor_scalar(out=m_[:], in0=ad[:], scalar1=1.0, scalar2=None,
                                    op0=Alu.is_lt)
            l_ = work.tile([128, 2048], dt.float32, tag="t_py")
            nc.vector.tensor_scalar_add(l_[:], ad[:], -0.5)
            qml = work.tile([128, 2048], dt.float32, tag="t_qy")
            nc.vector.scalar_tensor_tensor(out=qml[:], in0=q[:], scalar=0.5,
                                           in1=l_[:], op0=Alu.mult, op1=Alu.subtract)
            nc.gpsimd.tensor_tensor(out=m_[:], in0=m_[:], in1=qml[:], op=Alu.mult)
            vsl = work.tile([128, 2048], dt.float32, tag=f"t_sl{r}")
            nc.gpsimd.tensor_tensor(out=vsl[:], in0=l_[:], in1=m_[:], op=Alu.add)
            vsl_r.append(vsl)

        # ================= forced-anchor one-hot rescan (PE-paced) ==========
        eqf_tiles = [teq, tiou]
        for c in range(NCHUNK):
            for r in range(R):
                k = c * R + r
                gsh = s2[:, 68 + k:69 + k]
                nc.vector.tensor_scalar_add(gsh, gaidx[:, r:r + 1], float(-c * CH))
                eqf = work.tile([128, CH], dt.float32,
                                tag=["t_eq", "t_iou"][k % 2])
                nc.vector.tensor_scalar(out=eqf[:], in0=iotaf[:], scalar1=gsh,
                                        scalar2=None, op0=Alu.is_equal)
                fc_ps = psB.tile([128, FC, 2], dt.float32, tag="mmp")
                for t in range(FC):
                    nc.tensor.matmul(fc_ps[:, t, :], eqf[:, t * 128:(t + 1) * 128],
                                     indc[:], start=True, stop=True)
                nc.vector.tensor_copy(fcnt[:, r, c, :, :], fc_ps[:])

        # ================= pos / npos (needs fcnt) =================
        rm_flat = rm[:].rearrange("p r c i t -> p r c t i")
        fc_flat = fcnt.rearrange("p r c t i -> p (r c t i)")
        pos = state[:, 0, :]
        thr = work.tile([128, 1024], dt.float32, tag="t_qy")
        nc.vector.tensor_scalar(out=thr[:], in0=rm_flat, scalar1=0.5, scalar2=None,
                                op0=Alu.is_gt)
        nc.vector.scalar_tensor_tensor(out=pos, in0=fc_flat, scalar=1.0, in1=thr[:],
                                       op0=Alu.is_ge, op1=Alu.max)
        np4 = s2[:, 148:152]
        nc.vector.tensor_reduce(
            out=np4,
            in_=pos.rearrange("p (r c t i) -> p r i c t", r=R, c=NCHUNK, t=FC),
            axis=mybir.AxisListType.XY, op=Alu.add)
        np4t = s2[:, 152:156]
        nc.gpsimd.partition_all_reduce(np4t, np4, channels=128,
                                       reduce_op=bass_isa.ReduceOp.add)
        nn4 = s2[:, 156:160]
        t3 = s2[:, 160:164]
        nc.vector.tensor_scalar_mul(t3, np4t, NEG_POS_RATIO)
        rem = s2[:, 164:168]
        nc.vector.tensor_scalar(out=rem, in0=np4t, scalar1=-1.0, scalar2=float(A),
                                op0=Alu.mult, op1=Alu.add)
        nc.vector.tensor_tensor(out=nn4, in0=t3, in1=rem, op=Alu.min)

        # ================= bce masked sums =================
        pc1 = s2[:, 168:169]
        scr3 = work.tile([128, 1024], dt.float32, tag="t_aga")
        nc.vector.scalar_tensor_tensor(out=scr3[:], in0=logp, scalar=-1.0, in1=pos,
                                       op0=Alu.mult, op1=Alu.mult, accum_out=pc1)
        negl = state[:, 1, :]   # overwrites fcnt (already consumed)
        nc.vector.scalar_tensor_tensor(out=negl, in0=pos, scalar=1.0, in1=l1m,
                                       op0=Alu.subtract, op1=Alu.mult)

        # ================= loc masked sums =================
        la = s2[:, 169:171]
        for r in range(R):
            pos_r = state[:, 0, r * 512:(r + 1) * 512]
            pos_ap = _AP(tensor=state.tensor, offset=pos_r.offset,
                         ap=[pos_r.ap[0], [0, 4], [1, 2], [2, F]])
            posb = work.tile([128, 4, 2, F], dt.float32, tag="t_py")
            nc.vector.tensor_copy(posb[:], pos_ap)
            posbf = posb[:].rearrange("p c i f -> p (c i f)")
            sc4 = work.tile([128, 2048], dt.float32, tag="t_qy")
            nc.vector.scalar_tensor_tensor(out=sc4[:], in0=vsl_r[r][:], scalar=1.0,
                                           in1=posbf, op0=Alu.mult, op1=Alu.mult,
                                           accum_out=la[:, r:r + 1])

        # ================= hard-negative bisection =================
        # nn2[img] = 2*nn - 32768 for the ACT sign-counted images (img 0,1);
        # nn12 keeps plain nn for the DVE-counted images (img 2,3).
        nnmix = s2[:, 176:188]
        for j in range(3):
            nc.vector.tensor_copy(nnmix[:, j * 4:(j + 1) * 4], nn4)
        lo = s2[:, 188:192]
        nc.vector.memset(lo, 0.0)
        negl_v = state[:, 1, :].rearrange("p (r f i) -> p r f i", r=R, f=F)
        delta = BIS_RANGE
        for rnd in range(BIS_ROUNDS):
            thrT = s2[:, 192:204]
            for j in range(3):
                nc.vector.tensor_scalar_add(thrT[:, j * 4:(j + 1) * 4], lo,
                                            (j + 1) * delta / 4.0)
            cnt12 = s2[:, 216:228]
            for j in range(3):
                for r in range(R):
                    for i in range(2):
                        img = r * 2 + i
                        k = j * 4 + img
                        msk = work.tile([128, F], dt.float32, tag="t_qy")
                        nc.vector.tensor_scalar(
                            out=msk[:], in0=negl_v[:, r, :, i],
                            scalar1=thrT[:, k:k + 1],
                            scalar2=None, op0=Alu.is_gt, op1=Alu.add,
                            accum_out=cnt12[:, k:k + 1])
            ct_ps = psB.tile([128, 12], dt.float32, tag="mmp")
            nc.tensor.matmul(ct_ps[:], ones128[:], cnt12, start=True, stop=True)
            ge12 = s2[:, 228:240]
            nc.vector.tensor_tensor(out=ge12, in0=ct_ps[:], in1=nnmix, op=Alu.is_ge)
            s4 = s2[:, 240:244]
            ge_v = _AP(tensor=state.tensor, offset=ge12.offset,
                       ap=[ge12.ap[0], [1, 4], [4, 3]])
            nc.vector.tensor_reduce(out=s4, in_=ge_v, axis=X, op=Alu.add)
            lo_new = s2[:, 244 + rnd * 4:248 + rnd * 4]
            nc.vector.scalar_tensor_tensor(out=lo_new, in0=s4, scalar=delta / 4.0,
                                           in1=lo, op0=Alu.mult, op1=Alu.add)
            lo = lo_new
            delta = delta / 4.0
        tfin = s2[:, 296:300]
        nc.vector.tensor_scalar_add(tfin, lo, delta)
        cntf = s2[:, 300:304]
        svf = s2[:, 304:308]
        for r in range(R):
            for i in range(2):
                img = r * 2 + i
                msk = work.tile([128, F], dt.float32, tag="t_qy")
                nc.vector.tensor_scalar(
                    out=msk[:], in0=negl_v[:, r, :, i],
                    scalar1=tfin[:, img:img + 1], scalar2=None,
                    op0=Alu.is_gt, op1=Alu.add,
                    accum_out=cntf[:, img:img + 1])
                sv = work.tile([128, F], dt.float32, tag="t_px")
                nc.vector.scalar_tensor_tensor(
                    out=sv[:], in0=negl_v[:, r, :, i], scalar=1.0, in1=msk[:],
                    op0=Alu.mult, op1=Alu.mult, accum_out=svf[:, img:img + 1])

        # ================= final reduce & output =================
        part = s2[:, 320:336]
        nc.vector.memset(part[:, 3:5], 0.0)
        nc.vector.memset(part[:, 13:16], 0.0)
        nc.vector.tensor_copy(part[:, 0:1], pc1)
        nc.vector.tensor_copy(part[:, 1:2], la[:, 0:1])
        nc.vector.tensor_copy(part[:, 2:3], la[:, 1:2])
        nc.vector.tensor_copy(part[:, 5:9], cntf)
        nc.vector.tensor_copy(part[:, 9:13], svf)
        tot = s2[:, 336:352]
        nc.gpsimd.partition_all_reduce(tot, part, channels=128,
                                       reduce_op=bass_isa.ReduceOp.add)
        fin = state[0:1, 2, 352:384]
        # loc = la0+la1+lb0+lb1
        nc.vector.tensor_tensor(out=fin[:, 16:18], in0=tot[0:1, 1:3],
                                in1=tot[0:1, 3:5], op=Alu.add)
        nc.vector.tensor_reduce(out=fin[:, 0:1], in_=fin[:, 16:18], axis=X, op=Alu.add)
        nc.vector.tensor_copy(fin[:, 1:2], tot[0:1, 0:1])
        nc.vector.tensor_tensor(out=fin[:, 20:24], in0=nn4[0:1, :],
                                in1=tot[0:1, 5:9], op=Alu.subtract)
        nc.vector.tensor_tensor(out=fin[:, 24:28], in0=fin[:, 20:24],
                                in1=tfin[0:1, :], op=Alu.mult)
        nc.vector.tensor_tensor(out=fin[:, 28:32], in0=fin[:, 24:28],
                                in1=tot[0:1, 9:13], op=Alu.add)
        nc.vector.tensor_reduce(out=fin[:, 2:3], in_=fin[:, 28:32], axis=X, op=Alu.add)
        nc.vector.tensor_reduce(out=fin[:, 3:4], in_=np4t[0:1, :], axis=X, op=Alu.add)
        nc.vector.memset(fin[:, 4:8], 0.0)
        outt = consts.tile([1, 8], dt.float32)
        nc.vector.tensor_copy(outt[:], fin[:, 0:8])
        nc.sync.dma_start(out_d[:], outt[:])
        ctx.close()

    nc.compile()
    names = dict(anch_rows=anch_rows.name, anch_ap=anch_ap.name, gt_cols=gt_cols.name,
                 bbox_ap=bbox_in.name, conf_ap=conf_in.name, out=out_d.name)
    return nc, names


def get_program():
    if "prog" not in _CACHE:
        _CACHE["prog"] = _build_program()
    return _CACHE["prog"]


def make_core_inputs(bbox_pred, conf_pred, anchors, gt_boxes, core, names):
    i0 = core * IMGS
    bb = np.ascontiguousarray(
        bbox_pred[i0:i0 + IMGS].reshape(R, 2, F, 128, 4).transpose(3, 0, 2, 1, 4))
    cf = np.ascontiguousarray(
        conf_pred[i0:i0 + IMGS].reshape(R, 2, F, 128).transpose(3, 0, 2, 1))
    gt = np.ascontiguousarray(gt_boxes[i0:i0 + IMGS].reshape(R, 128, 4))
    ar = np.ascontiguousarray(anchors.T)
    aap = np.ascontiguousarray(anchors.reshape(F, 128, 4).transpose(1, 2, 0))
    return {names["anch_rows"]: ar.astype(np.float32),
            names["anch_ap"]: aap.astype(np.float32),
            names["gt_cols"]: gt.astype(np.float32),
            names["bbox_ap"]: bb.astype(np.float32),
            names["conf_ap"]: cf.astype(np.float32)}


def combine_partials(parts):
    p = np.stack([np.asarray(x).reshape(8) for x in parts]).astype(np.float32)
    loc = np.float32(p[:, 0].sum())
    pconf = np.float32(p[:, 1].sum())
    negc = np.float32(p[:, 2].sum())
    npos = np.float32(p[:, 3].sum())
    total_pos = np.float32(max(1.0, npos))
    loc_loss = np.float32(loc / total_pos)
    conf_loss = np.float32((pconf + negc) / total_pos)
    return (np.float32(loc_loss + conf_loss), conf_loss, loc_loss)


def kernel(bbox_pred, conf_pred, anchors, gt_boxes):
    from concourse.bass_utils import run_bass_kernel_spmd
    nc, names = get_program()
    in_maps = [make_core_inputs(bbox_pred, conf_pred, anchors, gt_boxes, k, names)
               for k in range(N_CORES)]
    res = run_bass_kernel_spmd(nc, in_maps, core_ids=list(range(N_CORES)))
    parts = [res.results[k][names["out"]] for k in range(N_CORES)]
    return combine_partials(parts)
